# revision 13
# baseline (speedup 1.0000x reference)
"""NT-Xent loss on 8 Trainium2 NeuronCores.

Math (reference): z = concat(z_i, z_j) [4096, 512]; zn = z / max(||z||, 1e-8);
sim = (zn @ zn.T) / 0.5; pos_i = sim[i, (i+2048) % 4096];
loss = mean_i( log(exp(pos_i) + sum_{j != i} exp(sim_ij)) - pos_i ).

Sharding: data-parallel over row-blocks of sim. Core c receives
zp_c = roll(z, -512*c, axis=0) so that every core runs the IDENTICAL program:
its own 512 rows sit at local rows 0..511, the positive partners at local
rows 2048..2559, and the diagonal of row-tile m lands at local cols
[128m, 128m+128). Row sums are rotation-invariant. Host sums the 8 per-core
[128, 4] partial contributions and divides by 4096.

Temperature fold: s = sqrt(2) * zn so that s @ s.T = sim (no /0.5 needed).
Transposes are done on the PE by multiplying each z row-tile with
diag(sqrt(2)/||z||) built from the identity, which also performs the
normalization; the result is cast to bf16 for the main matmul
(validated: loss rel err ~7e-7 vs fp32 reference).
"""

import numpy as np

try:
    import concourse.bass as bass
except ImportError:
    import sys
    sys.path.insert(0, "/opt/trn_rl_repo")
    import concourse.bass as bass

import concourse.bacc as bacc
import concourse.mybir as mybir
from concourse.tile import TileContext
from concourse.bass_utils import run_bass_kernel_spmd

N = 4096          # total rows (2*B)
D = 512           # feature dim
S = 512           # rows per core
NCORES = 8

TRACE = False
LAST_EXEC_NS = None
LAST_RESULTS = None

_NC_CACHE = None


def _build():
    fp32 = mybir.dt.float32
    bf16 = mybir.dt.bfloat16
    AF = mybir.ActivationFunctionType
    ALU = mybir.AluOpType
    AX = mybir.AxisListType

    nc = bacc.Bacc("TRN2")
    zp = nc.dram_tensor("zp", [N, D], fp32, kind="ExternalInput")
    ident = nc.dram_tensor("ident", [128, 128], fp32, kind="ExternalInput")
    out = nc.dram_tensor("out", [128, 4], fp32, kind="ExternalOutput")

    with TileContext(nc) as tc:
        with tc.tile_pool(name="persist", bufs=1) as pers, \
             tc.tile_pool(name="sqp", bufs=2) as sqp, \
             tc.tile_pool(name="dgp", bufs=3) as dgp, \
             tc.tile_pool(name="exp", bufs=3) as exp_pool, \
             tc.tile_pool(name="ttrp", bufs=2) as ttrp, \
             tc.tile_pool(name="psT", bufs=4, space="PSUM") as ptp, \
             tc.tile_pool(name="psM", bufs=2, space="PSUM") as pmp:

            idn = pers.tile([128, 128], fp32, name="idn")
            z_s = [pers.tile([128, D], fp32, name=f"z{t}") for t in range(32)]
            # znT[k][dsub, local_row] = z[local_row, 128k+dsub] * sqrt(2)/norm
            znT = [pers.tile([128, N], bf16, name=f"znT{k}") for k in range(4)]
            sumsq = pers.tile([128, 32], fp32, name="sumsq")
            nrm = pers.tile([128, 32], fp32, name="nrm")
            nrmc = pers.tile([128, 32], fp32, name="nrmc")
            inv = pers.tile([128, 32], fp32, name="inv")
            rowsum = pers.tile([128, 32], fp32, name="rowsum")
            diag = pers.tile([128, 4], fp32, name="diagv")
            posv = pers.tile([128, 4], fp32, name="posv")
            expd = pers.tile([128, 4], fp32, name="expd")
            expp = pers.tile([128, 4], fp32, name="expp")
            tot = pers.tile([128, 4], fp32, name="tot")
            sume = pers.tile([128, 4], fp32, name="sume")
            sume2 = pers.tile([128, 4], fp32, name="sume2")
            lse = pers.tile([128, 4], fp32, name="lse")
            res = pers.tile([128, 4], fp32, name="res")

            nc.sync.dma_start(out=idn[:], in_=ident[:])
            for t in range(32):
                nc.sync.dma_start(out=z_s[t][:], in_=zp[t * 128:(t + 1) * 128, :])

            # row sumsq via ACT Square with free-dim accumulate
            for t in range(32):
                sq = sqp.tile([128, D], fp32, name="sq")
                nc.scalar.activation(sq[:], z_s[t][:], AF.Square,
                                     accum_out=sumsq[:, t:t + 1])

            # nrm = sqrt(sumsq*0.5) = norm/sqrt(2); inv = sqrt(2)/max(norm,1e-8)
            nc.scalar.activation(nrm[:], sumsq[:], AF.Sqrt, scale=0.5)
            nc.vector.tensor_scalar_max(nrmc[:], nrm[:], 1e-8 * 0.7071067811865476)
            nc.vector.reciprocal(inv[:], nrmc[:])

            # transpose + normalize: psum = z_tile[:, k128].T @ diag(inv_t)
            for t in range(32):
                dg = dgp.tile([128, 128], fp32, name="dg")
                nc.gpsimd.tensor_scalar_mul(dg[:], idn[:], inv[:, t:t + 1])
                for k in range(4):
                    pt = ptp.tile([128, 128], fp32, name="pt")
                    nc.tensor.matmul(pt[:], z_s[t][:, k * 128:(k + 1) * 128], dg[:],
                                     start=True, stop=True)
                    nc.vector.tensor_copy(znT[k][:, t * 128:(t + 1) * 128], pt[:])

            # sim row-block: for each own row-tile m, 8 psum chunks of 512 cols
            for m in range(4):
                for n in range(8):
                    pm = pmp.tile([128, 512], fp32, name="pm")
                    for k in range(4):
                        nc.tensor.matmul(pm[:], znT[k][:, m * 128:(m + 1) * 128],
                                         znT[k][:, n * 512:(n + 1) * 512],
                                         start=(k == 0), stop=(k == 3))
                    ex = exp_pool.tile([128, 512], bf16, name="ex")
                    nc.scalar.activation(ex[:], pm[:], AF.Exp,
                                         accum_out=rowsum[:, m * 8 + n:m * 8 + n + 1])
                    if n in (0, 4):
                        # diag of pm[:, 128m:128m+128] (n=0 -> self, n=4 -> positive)
                        # via mask-mult + row reduce (fused ttr faults on HW)
                        to = ttrp.tile([128, 128], fp32, name="to")
                        tgt = diag if n == 0 else posv
                        nc.vector.tensor_mul(to[:], pm[:, m * 128:(m + 1) * 128],
                                             idn[:])
                        nc.vector.tensor_reduce(tgt[:, m:m + 1], to[:],
                                                AX.X, ALU.add)

            # loss_i = log(rowsum_i - exp(diag_i) + exp(pos_i)) - pos_i
            nc.scalar.activation(expd[:], diag[:], AF.Exp)
            nc.scalar.activation(expp[:], posv[:], AF.Exp)
            for m in range(4):
                nc.vector.tensor_reduce(tot[:, m:m + 1], rowsum[:, m * 8:(m + 1) * 8],
                                        AX.X, ALU.add)
            nc.vector.tensor_sub(sume[:], tot[:], expd[:])
            nc.vector.tensor_add(sume2[:], sume[:], expp[:])
            nc.scalar.activation(lse[:], sume2[:], AF.Ln)
            nc.vector.tensor_sub(res[:], lse[:], posv[:])
            nc.sync.dma_start(out=out[:], in_=res[:])

    nc.finalize()
    return nc


def kernel(**inputs):
    global _NC_CACHE, LAST_EXEC_NS, LAST_RESULTS
    z = np.concatenate([np.asarray(inputs["z_i"], dtype=np.float32),
                        np.asarray(inputs["z_j"], dtype=np.float32)], axis=0)
    ident = np.eye(128, dtype=np.float32)
    if _NC_CACHE is None:
        _NC_CACHE = _build()
    in_maps = [{"zp": np.roll(z, -S * c, axis=0), "ident": ident}
               for c in range(NCORES)]
    br = run_bass_kernel_spmd(_NC_CACHE, in_maps, list(range(NCORES)),
                              trace=TRACE)
    LAST_RESULTS = br
    LAST_EXEC_NS = getattr(br, "exec_time_ns", None)
    total = np.float64(0.0)
    for r in br.results:
        total += np.asarray(r["out"], dtype=np.float64).sum()
    return np.asarray(total / N, dtype=np.float32)



# revision 14
# speedup vs baseline: 1.0790x; 1.0790x over previous
"""NT-Xent loss on 8 Trainium2 NeuronCores — v2 (optimized).

Same math/sharding as v1 (host roll makes all cores run an identical program;
core c's rows at local 0..511, positives at local 2048..2559). v2 changes:
- No GpSimd diag tiles, no fp32 PE transposes. z is scaled by sqrt(2)/norm on
  DVE (bf16 out), stored to DRAM scratch, and transposed back into [d, row]
  layout with 8 XBAR dma_start_transpose ops (DMA engines, not PE).
- Squares for row norms: 1/4 on ACT (Square+accum), 3/4 on DVE
  (scalar_tensor_tensor z*z with fused accum) to balance engines.
- Main matmul accumulates 4x512-col chunks into a [128, 2048] PSUM tile
  (4 banks); one wide Exp+accum per tile (8 total instead of 32).
- Input DMAs batched 4 row-tiles at a time via 3D APs (8 DMAs instead of 32).
"""

import numpy as np

try:
    import concourse.bass as bass
except ImportError:
    import sys
    sys.path.insert(0, "/opt/trn_rl_repo")
    import concourse.bass as bass

import concourse.bacc as bacc
import concourse.mybir as mybir
from concourse.tile import TileContext
from concourse.bass_utils import run_bass_kernel_spmd

N = 4096          # total rows (2*B)
D = 512           # feature dim
S = 512           # rows per core
NCORES = 8

DVE_SQUARES = True   # use DVE scalar_tensor_tensor for 3 of 4 squares per group

TRACE = False
LAST_EXEC_NS = None
LAST_RESULTS = None

_NC_CACHE = None


def _build():
    fp32 = mybir.dt.float32
    bf16 = mybir.dt.bfloat16
    AF = mybir.ActivationFunctionType
    ALU = mybir.AluOpType
    AX = mybir.AxisListType

    nc = bacc.Bacc("TRN2")
    zp = nc.dram_tensor("zp", [N, D], fp32, kind="ExternalInput")
    identf = nc.dram_tensor("identf", [128, 128], fp32, kind="ExternalInput")
    out = nc.dram_tensor("out", [128, 4], fp32, kind="ExternalOutput")

    with TileContext(nc) as tc:
        with tc.tile_pool(name="persist", bufs=1) as pers, \
             tc.tile_pool(name="sqp", bufs=3) as sqp, \
             tc.tile_pool(name="exp", bufs=2) as exp_pool, \
             tc.tile_pool(name="ttrp", bufs=2) as ttrp, \
             tc.tile_pool(name="dram", bufs=1, space="DRAM") as dpool, \
             tc.tile_pool(name="psM", bufs=2, space="PSUM") as pmp:

            idn = pers.tile([128, 128], fp32, name="idn")
            zbig = [pers.tile([128, 2048], fp32, name=f"zb{g}") for g in range(8)]
            zsb = [pers.tile([128, 2048], bf16, name=f"zs{g}") for g in range(8)]
            # znT[k][d, r] = zscl[r, 128k+d]
            znT = [pers.tile([128, N], bf16, name=f"znT{k}") for k in range(4)]
            zscl_d = dpool.tile([N, D], bf16, name="zscl_d")

            sumsq = pers.tile([128, 32], fp32, name="sumsq")
            nrm = pers.tile([128, 32], fp32, name="nrm")
            nrmc = pers.tile([128, 32], fp32, name="nrmc")
            inv = pers.tile([128, 32], fp32, name="inv")
            rowsum = pers.tile([128, 8], fp32, name="rowsum")
            diag = pers.tile([128, 4], fp32, name="diagv")
            posv = pers.tile([128, 4], fp32, name="posv")
            expd = pers.tile([128, 4], fp32, name="expd")
            expp = pers.tile([128, 4], fp32, name="expp")
            tot = pers.tile([128, 4], fp32, name="tot")
            sume = pers.tile([128, 4], fp32, name="sume")
            sume2 = pers.tile([128, 4], fp32, name="sume2")
            lse = pers.tile([128, 4], fp32, name="lse")
            res = pers.tile([128, 4], fp32, name="res")

            nc.sync.dma_start(out=idn[:], in_=identf[:])
            for g in range(8):
                nc.sync.dma_start(
                    out=zbig[g][:].rearrange("p (j d) -> p j d", j=4),
                    in_=zp[g * 512:(g + 1) * 512, :].rearrange(
                        "(j p) d -> p j d", p=128))

            for g in range(8):
                for j in range(4):
                    t = g * 4 + j
                    zsl = zbig[g][:, j * 512:(j + 1) * 512]
                    sq = sqp.tile([128, 512], fp32, name="sq")
                    if DVE_SQUARES and j > 0:
                        nc.vector.scalar_tensor_tensor(
                            sq[:], zsl, 1.0, zsl, ALU.mult, ALU.mult,
                            accum_out=sumsq[:, t:t + 1])
                    else:
                        nc.scalar.activation(sq[:], zsl, AF.Square,
                                             accum_out=sumsq[:, t:t + 1])
                # inv = sqrt(2)/max(norm, 1e-8); nrm = norm/sqrt(2)
                g4 = g * 4
                nc.scalar.activation(nrm[:, g4:g4 + 4], sumsq[:, g4:g4 + 4],
                                     AF.Sqrt, scale=0.5)
                nc.vector.tensor_scalar_max(nrmc[:, g4:g4 + 4], nrm[:, g4:g4 + 4],
                                            1e-8 * 0.7071067811865476)
                nc.vector.reciprocal(inv[:, g4:g4 + 4], nrmc[:, g4:g4 + 4])
                for j in range(4):
                    t = g * 4 + j
                    nc.vector.tensor_scalar_mul(
                        zsb[g][:, j * 512:(j + 1) * 512],
                        zbig[g][:, j * 512:(j + 1) * 512], inv[:, t:t + 1])
                nc.sync.dma_start(
                    out=zscl_d[g * 512:(g + 1) * 512, :].rearrange(
                        "(j p) d -> p j d", p=128),
                    in_=zsb[g][:].rearrange("p (j d) -> p j d", j=4))
                if g == 3:
                    for k in range(4):
                        nc.sync.dma_start_transpose(
                            out=znT[k][:, 0:2048],
                            in_=zscl_d[0:2048, k * 128:(k + 1) * 128])
                if g == 7:
                    for k in range(4):
                        nc.sync.dma_start_transpose(
                            out=znT[k][:, 2048:4096],
                            in_=zscl_d[2048:4096, k * 128:(k + 1) * 128])

            # sim row-block: per (half, m) a [128, 2048] psum tile (4 n-chunks)
            for half in range(2):
                for m in range(4):
                    pm = pmp.tile([128, 2048], fp32, name="pm")
                    for nn in range(4):
                        n = half * 4 + nn
                        for k in range(4):
                            nc.tensor.matmul(
                                pm[:, nn * 512:(nn + 1) * 512],
                                znT[k][:, m * 128:(m + 1) * 128],
                                znT[k][:, n * 512:(n + 1) * 512],
                                start=(k == 0), stop=(k == 3))
                    ex = exp_pool.tile([128, 2048], bf16, name="ex")
                    idx = half * 4 + m
                    nc.scalar.activation(ex[:], pm[:], AF.Exp,
                                         accum_out=rowsum[:, idx:idx + 1])
                    # diag (half 0) / positive (half 1) sits in nn=0 chunk at
                    # cols m*128; mask-mult + row reduce (fused ttr faults on HW)
                    to = ttrp.tile([128, 128], fp32, name="to")
                    tgt = diag if half == 0 else posv
                    nc.vector.tensor_mul(to[:], pm[:, m * 128:(m + 1) * 128],
                                         idn[:])
                    nc.vector.tensor_reduce(tgt[:, m:m + 1], to[:], AX.X, ALU.add)

            # loss_i = log(rowsum_i - exp(diag_i) + exp(pos_i)) - pos_i
            nc.scalar.activation(expd[:], diag[:], AF.Exp)
            nc.scalar.activation(expp[:], posv[:], AF.Exp)
            nc.vector.tensor_add(tot[:], rowsum[:, 0:4], rowsum[:, 4:8])
            nc.vector.tensor_sub(sume[:], tot[:], expd[:])
            nc.vector.tensor_add(sume2[:], sume[:], expp[:])
            nc.scalar.activation(lse[:], sume2[:], AF.Ln)
            nc.vector.tensor_sub(res[:], lse[:], posv[:])
            nc.sync.dma_start(out=out[:], in_=res[:])

    nc.finalize()
    return nc


def kernel(**inputs):
    global _NC_CACHE, LAST_EXEC_NS, LAST_RESULTS
    z = np.concatenate([np.asarray(inputs["z_i"], dtype=np.float32),
                        np.asarray(inputs["z_j"], dtype=np.float32)], axis=0)
    ident = np.eye(128, dtype=np.float32)
    if _NC_CACHE is None:
        _NC_CACHE = _build()
    in_maps = [{"zp": np.roll(z, -S * c, axis=0), "identf": ident}
               for c in range(NCORES)]
    br = run_bass_kernel_spmd(_NC_CACHE, in_maps, list(range(NCORES)),
                              trace=TRACE)
    LAST_RESULTS = br
    LAST_EXEC_NS = getattr(br, "exec_time_ns", None)
    total = np.float64(0.0)
    for r in br.results:
        total += np.asarray(r["out"], dtype=np.float64).sum()
    return np.asarray(total / N, dtype=np.float32)
